# revision 24
# baseline (speedup 1.0000x reference)
"""Trainium2 Bass kernel for nn_Attention_558345749040.

Reference computation (per batch b, H=8 heads of d=64, S=4096, E=512):
    Q = Q_seq @ WQ ; K = K_seq @ WK ; V = V_seq @ WV      (per-token matmuls)
    A = (Q * K) / 8                                        (elementwise)
    A += -1e12 at head positions j >= V_len[b]             (additive mask)
    softmax over each head's 64-wide feature group
    O = softmax * V, rows s >= Q_len[b] zeroed

Sharding: pure data parallel, batch b -> core b (B == 8 == n_cores).

Device algorithm (per core, token-major [128-token, 512-feature] tiles):
  Q/K projections in float32r (full-rate PE, enough mantissa for the exp),
  V projection in fp16. Host pre-zeroes masked columns of WK and WV, so
  masked positions have K=0 => logits A_j = 0 exactly; the mask-free group
  max is then >= 0 and >= every unmasked logit, making exp(A - M) <= 1 and
  leaving masked positions excluded from the denominator via a 0/1 vmask
  multiply on exp's output (and zeroed in the output via the zeroed WV).
  V_len == 0 cores reproduce the reference's uniform-1/64 softmax via
  WK = 0 with vmask = 1. Q_len row masking rides the V PSUM->SBUF copy as
  a per-partition ACT scale. The elementwise/softmax chain runs on wide
  [128, 1024] tiles (two token chunks per instruction) to amortize per-op
  overheads; matmul/PSUM stages stay per-chunk (PSUM bank budget).
"""

import numpy as np
import ml_dtypes

B, S, EMB = 8, 4096, 512
H, D = 8, 64
NCORES = 8
KC = EMB // 128          # 4 contraction chunks
NCHUNK = S // 128        # 32 token chunks
SUP = 4                  # token chunks per super-chunk (input DMA granularity)
NSUP = NCHUNK // SUP
W = 4                    # token chunks per wide elementwise tile

_CACHE = {}


def _build(cfg=""):
    import concourse.bacc as bacc
    import concourse.mybir as mybir
    from concourse.tile import TileContext

    f32 = mybir.dt.float32
    f32r = mybir.dt.float32r
    f16 = mybir.dt.float16
    bf16 = mybir.dt.bfloat16
    AX = mybir.AxisListType
    OP = mybir.AluOpType
    ACTF = mybir.ActivationFunctionType

    nc = bacc.Bacc()

    WE = W * EMB
    qT = nc.declare_dram_parameter("qT", [EMB, S], f32r, isOutput=False)
    kT = nc.declare_dram_parameter("kT", [EMB, S], f32r, isOutput=False)
    vT = nc.declare_dram_parameter("vT", [EMB, S], f16, isOutput=False)
    wq = nc.declare_dram_parameter("wq", [EMB, EMB], f32r, isOutput=False)
    wk = nc.declare_dram_parameter("wk", [EMB, EMB], f32r, isOutput=False)
    wv = nc.declare_dram_parameter("wv", [EMB, EMB], f16, isOutput=False)
    vmask = nc.declare_dram_parameter("vmask", [128, WE], bf16, isOutput=False)
    qmask = nc.declare_dram_parameter("qmask", [128, NCHUNK], f32, isOutput=False)
    out = nc.declare_dram_parameter("out", [S, EMB], bf16, isOutput=True)

    def view_hd(ap):
        # [128, W*EMB] -> [128, W*H, D]
        return ap.rearrange("p (g d) -> p g d", d=D)

    def bcast_hd(ap):
        # [128, W*H] -> [128, W*H, D] with step-0 broadcast
        return ap.rearrange("p (g o) -> p g o", o=1).broadcast_to((128, W * H, D))

    with TileContext(nc) as tc:
        with (
            tc.tile_pool(name="consts", bufs=1) as cpool,
            tc.tile_pool(name="xin", bufs=2) as xpool,
            tc.tile_pool(name="ps", bufs=2, space="PSUM") as ppool,
            tc.tile_pool(name="psq3", bufs=3, space="PSUM") as qpool,
            tc.tile_pool(name="work", bufs=3) as wpool,
            tc.tile_pool(name="live", bufs=4) as lpool,
            tc.tile_pool(name="stats", bufs=4) as spool,
        ):
            w_sb = {}
            for name, src, dt_ in (("wq", wq, f32r), ("wk", wk, f32r),
                                   ("wv", wv, f16)):
                tiles = []
                for kc in range(KC):
                    t = cpool.tile([128, EMB], dt_, tag=f"{name}{kc}",
                                   name=f"{name}{kc}")
                    nc.sync.dma_start(out=t[:], in_=src[kc * 128:(kc + 1) * 128, :])
                    tiles.append(t)
                w_sb[name] = tiles
            vm_sb = cpool.tile([128, WE], bf16, tag="vmask")
            nc.sync.dma_start(out=vm_sb[:], in_=vmask[:, :])
            qm_sb = cpool.tile([128, NCHUNK], f32, tag="qm")
            nc.sync.dma_start(out=qm_sb[:], in_=qmask[:, :])

            npairs = NCHUNK // W

            def load_sup(s, split=1):
                # split>1: issue the load in `split` column slices so the
                # first pair's data lands early (kills the startup ramp).
                tok0 = s * SUP * 128
                cols = SUP * 128
                xs = {}
                tiles = {}
                for name, src, dt_ in (("q", qT, f32r), ("k", kT, f32r),
                                       ("v", vT, f16)):
                    tiles[name] = [xpool.tile([128, cols], dt_, tag=f"x{name}{kc}",
                                              name=f"x{name}{kc}")
                                   for kc in range(KC)]
                    xs[name] = tiles[name]
                for part in range(split):
                    c0, c1 = part * cols // split, (part + 1) * cols // split
                    for name, src, dt_ in (("q", qT, f32r), ("k", kT, f32r),
                                           ("v", vT, f16)):
                        for kc in range(KC):
                            nc.sync.dma_start(
                                out=tiles[name][kc][:, c0:c1],
                                in_=src[kc * 128:(kc + 1) * 128,
                                        tok0 + c0:tok0 + c1],
                            )
                return xs

            def stage_front(pair, xs):
                # matmuls, PSUM copies, logits, group max, max-subtract, exp
                k_sb = wpool.tile([128, WE], f32, tag="k_sb", bufs=2)
                v_sb = lpool.tile([128, WE], bf16, tag="v_sb")
                a = wpool.tile([128, WE], f32, tag="a", bufs=2)
                psvs = []
                for c in range(W):
                    chunk = pair * W + c
                    j = chunk % SUP
                    js = slice(j * 128, (j + 1) * 128)
                    cs = slice(c * EMB, (c + 1) * EMB)
                    psq = qpool.tile([128, EMB], f32, tag="psq")
                    psk = ppool.tile([128, EMB], f32, tag="psk")
                    for name, ps, wn in (("k", psk, "wk"), ("q", psq, "wq")):
                        for kc in range(KC):
                            nc.tensor.matmul(
                                ps[:],
                                xs[name][kc][:, js],
                                w_sb[wn][kc][:],
                                start=(kc == 0),
                                stop=(kc == KC - 1),
                            )
                    nc.scalar.copy(k_sb[:, cs], psk[:])
                    nc.vector.tensor_mul(a[:, cs], psq[:], k_sb[:, cs])
                for c in range(W):
                    chunk = pair * W + c
                    j = chunk % SUP
                    js = slice(j * 128, (j + 1) * 128)
                    cs = slice(c * EMB, (c + 1) * EMB)
                    psv = ppool.tile([128, EMB], f32, tag="psv", bufs=3)
                    for kc in range(KC):
                        nc.tensor.matmul(
                            psv[:],
                            xs["v"][kc][:, js],
                            w_sb["wv"][kc][:],
                            start=(kc == 0),
                            stop=(kc == KC - 1),
                        )
                    nc.scalar.activation(
                        v_sb[:, cs], psv[:], ACTF.Copy,
                        scale=qm_sb[:, chunk:chunk + 1],
                    )
                mneg = spool.tile([128, W * H], f32, tag="mneg")
                nc.vector.tensor_reduce(
                    mneg[:], view_hd(a[:]), axis=AX.X, op=OP.max, negate=True
                )
                t_m = wpool.tile([128, WE], f32, tag="t_m", bufs=2)
                nc.gpsimd.tensor_add(
                    view_hd(t_m[:]), view_hd(a[:]), bcast_hd(mneg[:])
                )
                e = lpool.tile([128, WE], bf16, tag="e")
                nc.scalar.activation(e[:], t_m[:], ACTF.Exp)
                return e, v_sb

            def stage_back(pair, e, v_sb):
                # denominator, reciprocal, normalize, weight V, store
                em = wpool.tile([128, WE], bf16, tag="em")
                nc.vector.tensor_mul(em[:], e[:], vm_sb[:])
                ssum = spool.tile([128, W * H], f32, tag="ssum")
                nc.vector.tensor_reduce(
                    ssum[:], view_hd(em[:]), axis=AX.X, op=OP.add
                )
                r = spool.tile([128, W * H], bf16, tag="r")
                with nc.allow_low_precision(reason="1/S at bf16: 0.4% on softmax weights, well under the 2e-2 gate"):
                    nc.vector.reciprocal(r[:], ssum[:])
                p = wpool.tile([128, WE], bf16, tag="p")
                nc.gpsimd.tensor_mul(
                    view_hd(p[:]), view_hd(em[:]), bcast_hd(r[:])
                )
                o = wpool.tile([128, WE], bf16, tag="o")
                nc.vector.tensor_mul(o[:], p[:], v_sb[:])
                t0 = pair * W * 128
                nc.sync.dma_start(
                    out=out[t0:t0 + W * 128, :].rearrange("(i p) f -> p i f", i=W),
                    in_=o[:].rearrange("p (i f) -> p i f", i=W),
                )

            pairs_per_sup = SUP // W
            xs_cur = load_sup(0, split=1)
            xs_next = None
            pending = None
            for pair in range(npairs + 1):
                if pair < npairs:
                    s, local = divmod(pair, pairs_per_sup)
                    if local == 0 and s > 0:
                        xs_cur = load_sup(s)
                    front = stage_front(pair, xs_cur)
                else:
                    front = None
                if pending is not None:
                    stage_back(pair - 1, *pending)
                pending = front

    nc.finalize()
    return nc


def _prep_inputs(Q_seq, K_seq, V_seq, Q_len, V_len, WQ, WK, WV):
    in_maps = []
    jpos = np.arange(EMB) % D
    tpos = np.arange(S)
    for b in range(B):
        vl = int(V_len[b, 0])
        ql = int(Q_len[b, 0])
        if vl == 0:
            # Reference semantics collapse to a uniform 1/64 softmax (every
            # logit rides to exactly -1e12 in f32). Reproduce via K = 0
            # (all logits 0 -> uniform) with every position unmasked.
            wk_b = np.zeros_like(WK, dtype=np.float32)
            wv_b = WV.astype(np.float32)
            vmrow = np.ones(EMB, np.float32)
        else:
            keep = (jpos < vl)
            wk_b = np.where(keep[None, :], WK, 0.0).astype(np.float32)
            wv_b = np.where(keep[None, :], WV, 0.0).astype(np.float32)
            vmrow = keep.astype(np.float32)
        vmrow_w = np.tile(vmrow, W).astype(ml_dtypes.bfloat16)
        vmask = np.broadcast_to(vmrow_w, (128, W * EMB)).copy()
        qm = (tpos < ql).astype(np.float32).reshape(NCHUNK, 128).T.copy()
        in_maps.append({
            "qT": np.ascontiguousarray(Q_seq[b].T.astype(np.float32)),
            "kT": np.ascontiguousarray(K_seq[b].T.astype(np.float32)),
            "vT": np.ascontiguousarray(V_seq[b].T.astype(np.float16)),
            "wq": np.ascontiguousarray((WQ * 0.125).astype(np.float32)),
            "wk": np.ascontiguousarray(wk_b),
            "wv": np.ascontiguousarray(wv_b.astype(np.float16)),
            "vmask": vmask,
            "qmask": np.ascontiguousarray(qm),
        })
    return in_maps


def _run(inputs, trace=False, mm_dtype_name="", tmpdir=None):
    from concourse.bass_utils import run_bass_kernel_spmd

    key = "v7"
    if key not in _CACHE:
        _CACHE[key] = _build()
    nc = _CACHE[key]

    in_maps = _prep_inputs(**inputs)
    res = run_bass_kernel_spmd(nc, in_maps, core_ids=list(range(NCORES)),
                               trace=trace, tmpdir=tmpdir)
    out = np.stack([res.results[i]["out"] for i in range(NCORES)], axis=0)
    return out.astype(np.float32), res


def kernel(Q_seq, K_seq, V_seq, Q_len, V_len, WQ, WK, WV):
    out, _ = _run(dict(Q_seq=Q_seq, K_seq=K_seq, V_seq=V_seq,
                       Q_len=Q_len, V_len=V_len, WQ=WQ, WK=WK, WV=WV))
    return out


# revision 25
# speedup vs baseline: 1.0232x; 1.0232x over previous
"""Trainium2 Bass kernel for nn_Attention_558345749040.

Reference computation (per batch b, H=8 heads of d=64, S=4096, E=512):
    Q = Q_seq @ WQ ; K = K_seq @ WK ; V = V_seq @ WV      (per-token matmuls)
    A = (Q * K) / 8                                        (elementwise)
    A += -1e12 at head positions j >= V_len[b]             (additive mask)
    softmax over each head's 64-wide feature group
    O = softmax * V, rows s >= Q_len[b] zeroed

Sharding: pure data parallel, batch b -> core b (B == 8 == n_cores).

Device algorithm (per core, token-major [128-token, 512-feature] tiles):
  Q/K projections in float32r (full-rate PE, enough mantissa for the exp),
  V projection in fp16. Host pre-zeroes masked columns of WK and WV, so
  masked positions have K=0 => logits A_j = 0 exactly; the mask-free group
  max is then >= 0 and >= every unmasked logit, making exp(A - M) <= 1 and
  leaving masked positions excluded from the denominator via a 0/1 vmask
  multiply on exp's output (and zeroed in the output via the zeroed WV).
  V_len == 0 cores reproduce the reference's uniform-1/64 softmax via
  WK = 0 with vmask = 1. Q_len row masking rides the V PSUM->SBUF copy as
  a per-partition ACT scale. The elementwise/softmax chain runs on wide
  [128, 1024] tiles (two token chunks per instruction) to amortize per-op
  overheads; matmul/PSUM stages stay per-chunk (PSUM bank budget).
"""

import numpy as np
import ml_dtypes

B, S, EMB = 8, 4096, 512
H, D = 8, 64
NCORES = 8
KC = EMB // 128          # 4 contraction chunks
NCHUNK = S // 128        # 32 token chunks
SUP = 8                  # token chunks per super-chunk (input DMA granularity)
NSUP = NCHUNK // SUP
W = 2                    # token chunks per wide elementwise tile

_CACHE = {}


def _build(cfg=""):
    import concourse.bacc as bacc
    import concourse.mybir as mybir
    from concourse.tile import TileContext

    f32 = mybir.dt.float32
    f32r = mybir.dt.float32r
    f16 = mybir.dt.float16
    bf16 = mybir.dt.bfloat16
    AX = mybir.AxisListType
    OP = mybir.AluOpType
    ACTF = mybir.ActivationFunctionType

    nc = bacc.Bacc()

    WE = W * EMB
    qT = nc.declare_dram_parameter("qT", [EMB, S], f32r, isOutput=False)
    kT = nc.declare_dram_parameter("kT", [EMB, S], f32r, isOutput=False)
    vT = nc.declare_dram_parameter("vT", [EMB, S], f16, isOutput=False)
    wq = nc.declare_dram_parameter("wq", [EMB, EMB], f32r, isOutput=False)
    wk = nc.declare_dram_parameter("wk", [EMB, EMB], f32r, isOutput=False)
    wv = nc.declare_dram_parameter("wv", [EMB, EMB], f16, isOutput=False)
    vmask = nc.declare_dram_parameter("vmask", [128, WE], bf16, isOutput=False)
    qmask = nc.declare_dram_parameter("qmask", [128, NCHUNK], f32, isOutput=False)
    out = nc.declare_dram_parameter("out", [S, EMB], bf16, isOutput=True)

    def view_hd(ap):
        # [128, W*EMB] -> [128, W*H, D]
        return ap.rearrange("p (g d) -> p g d", d=D)

    def bcast_hd(ap):
        # [128, W*H] -> [128, W*H, D] with step-0 broadcast
        return ap.rearrange("p (g o) -> p g o", o=1).broadcast_to((128, W * H, D))

    with TileContext(nc) as tc:
        with (
            tc.tile_pool(name="consts", bufs=1) as cpool,
            tc.tile_pool(name="xin", bufs=2) as xpool,
            tc.tile_pool(name="ps", bufs=2, space="PSUM") as ppool,
            tc.tile_pool(name="psq3", bufs=3, space="PSUM") as qpool,
            tc.tile_pool(name="work", bufs=3) as wpool,
            tc.tile_pool(name="live", bufs=4) as lpool,
            tc.tile_pool(name="stats", bufs=4) as spool,
        ):
            w_sb = {}
            for name, src, dt_ in (("wq", wq, f32r), ("wk", wk, f32r),
                                   ("wv", wv, f16)):
                tiles = []
                for kc in range(KC):
                    t = cpool.tile([128, EMB], dt_, tag=f"{name}{kc}",
                                   name=f"{name}{kc}")
                    nc.sync.dma_start(out=t[:], in_=src[kc * 128:(kc + 1) * 128, :])
                    tiles.append(t)
                w_sb[name] = tiles
            vm_sb = cpool.tile([128, WE], bf16, tag="vmask")
            nc.sync.dma_start(out=vm_sb[:], in_=vmask[:, :])
            qm_sb = cpool.tile([128, NCHUNK], f32, tag="qm")
            nc.sync.dma_start(out=qm_sb[:], in_=qmask[:, :])

            npairs = NCHUNK // W

            def load_sup(s, split=1):
                # split>1: issue the load in `split` column slices so the
                # first pair's data lands early (kills the startup ramp).
                tok0 = s * SUP * 128
                cols = SUP * 128
                xs = {}
                tiles = {}
                for name, src, dt_ in (("q", qT, f32r), ("k", kT, f32r),
                                       ("v", vT, f16)):
                    tiles[name] = [xpool.tile([128, cols], dt_, tag=f"x{name}{kc}",
                                              name=f"x{name}{kc}")
                                   for kc in range(KC)]
                    xs[name] = tiles[name]
                for part in range(split):
                    c0, c1 = part * cols // split, (part + 1) * cols // split
                    for name, src, dt_ in (("q", qT, f32r), ("k", kT, f32r),
                                           ("v", vT, f16)):
                        for kc in range(KC):
                            nc.sync.dma_start(
                                out=tiles[name][kc][:, c0:c1],
                                in_=src[kc * 128:(kc + 1) * 128,
                                        tok0 + c0:tok0 + c1],
                            )
                return xs

            def stage_front(pair, xs):
                # matmuls, PSUM copies, logits, group max, max-subtract, exp
                k_sb = wpool.tile([128, WE], f32, tag="k_sb")
                v_sb = lpool.tile([128, WE], bf16, tag="v_sb")
                a = wpool.tile([128, WE], f32, tag="a")
                psvs = []
                for c in range(W):
                    chunk = pair * W + c
                    j = chunk % SUP
                    js = slice(j * 128, (j + 1) * 128)
                    cs = slice(c * EMB, (c + 1) * EMB)
                    psq = qpool.tile([128, EMB], f32, tag="psq")
                    psk = ppool.tile([128, EMB], f32, tag="psk")
                    for name, ps, wn in (("k", psk, "wk"), ("q", psq, "wq")):
                        for kc in range(KC):
                            nc.tensor.matmul(
                                ps[:],
                                xs[name][kc][:, js],
                                w_sb[wn][kc][:],
                                start=(kc == 0),
                                stop=(kc == KC - 1),
                            )
                    nc.scalar.copy(k_sb[:, cs], psk[:])
                    nc.vector.tensor_mul(a[:, cs], psq[:], k_sb[:, cs])
                for c in range(W):
                    chunk = pair * W + c
                    j = chunk % SUP
                    js = slice(j * 128, (j + 1) * 128)
                    cs = slice(c * EMB, (c + 1) * EMB)
                    psv = ppool.tile([128, EMB], f32, tag="psv", bufs=3)
                    for kc in range(KC):
                        nc.tensor.matmul(
                            psv[:],
                            xs["v"][kc][:, js],
                            w_sb["wv"][kc][:],
                            start=(kc == 0),
                            stop=(kc == KC - 1),
                        )
                    nc.scalar.activation(
                        v_sb[:, cs], psv[:], ACTF.Copy,
                        scale=qm_sb[:, chunk:chunk + 1],
                    )
                mneg = spool.tile([128, W * H], f32, tag="mneg")
                nc.vector.tensor_reduce(
                    mneg[:], view_hd(a[:]), axis=AX.X, op=OP.max, negate=True
                )
                t_m = wpool.tile([128, WE], f32, tag="t_m")
                nc.gpsimd.tensor_add(
                    view_hd(t_m[:]), view_hd(a[:]), bcast_hd(mneg[:])
                )
                e = lpool.tile([128, WE], bf16, tag="e")
                nc.scalar.activation(e[:], t_m[:], ACTF.Exp)
                return e, v_sb

            def stage_back(pair, e, v_sb):
                # denominator, reciprocal, normalize, weight V, store
                em = wpool.tile([128, WE], bf16, tag="em")
                nc.vector.tensor_mul(em[:], e[:], vm_sb[:])
                ssum = spool.tile([128, W * H], f32, tag="ssum")
                nc.vector.tensor_reduce(
                    ssum[:], view_hd(em[:]), axis=AX.X, op=OP.add
                )
                r = spool.tile([128, W * H], bf16, tag="r")
                with nc.allow_low_precision(reason="1/S at bf16: 0.4% on softmax weights, well under the 2e-2 gate"):
                    nc.vector.reciprocal(r[:], ssum[:])
                p = wpool.tile([128, WE], bf16, tag="p")
                nc.gpsimd.tensor_mul(
                    view_hd(p[:]), view_hd(em[:]), bcast_hd(r[:])
                )
                o = wpool.tile([128, WE], bf16, tag="o")
                nc.vector.tensor_mul(o[:], p[:], v_sb[:])
                t0 = pair * W * 128
                nc.sync.dma_start(
                    out=out[t0:t0 + W * 128, :].rearrange("(i p) f -> p i f", i=W),
                    in_=o[:].rearrange("p (i f) -> p i f", i=W),
                )

            pairs_per_sup = SUP // W
            xs_cur = load_sup(0, split=1)
            xs_next = None
            pending = None
            for pair in range(npairs + 1):
                if pair < npairs:
                    s, local = divmod(pair, pairs_per_sup)
                    if local == 0 and s > 0:
                        xs_cur = load_sup(s)
                    front = stage_front(pair, xs_cur)
                else:
                    front = None
                if pending is not None:
                    stage_back(pair - 1, *pending)
                pending = front

    nc.finalize()
    return nc


def _prep_inputs(Q_seq, K_seq, V_seq, Q_len, V_len, WQ, WK, WV):
    in_maps = []
    jpos = np.arange(EMB) % D
    tpos = np.arange(S)
    for b in range(B):
        vl = int(V_len[b, 0])
        ql = int(Q_len[b, 0])
        if vl == 0:
            # Reference semantics collapse to a uniform 1/64 softmax (every
            # logit rides to exactly -1e12 in f32). Reproduce via K = 0
            # (all logits 0 -> uniform) with every position unmasked.
            wk_b = np.zeros_like(WK, dtype=np.float32)
            wv_b = WV.astype(np.float32)
            vmrow = np.ones(EMB, np.float32)
        else:
            keep = (jpos < vl)
            wk_b = np.where(keep[None, :], WK, 0.0).astype(np.float32)
            wv_b = np.where(keep[None, :], WV, 0.0).astype(np.float32)
            vmrow = keep.astype(np.float32)
        vmrow_w = np.tile(vmrow, W).astype(ml_dtypes.bfloat16)
        vmask = np.broadcast_to(vmrow_w, (128, W * EMB)).copy()
        qm = (tpos < ql).astype(np.float32).reshape(NCHUNK, 128).T.copy()
        in_maps.append({
            "qT": np.ascontiguousarray(Q_seq[b].T.astype(np.float32)),
            "kT": np.ascontiguousarray(K_seq[b].T.astype(np.float32)),
            "vT": np.ascontiguousarray(V_seq[b].T.astype(np.float16)),
            "wq": np.ascontiguousarray((WQ * 0.125).astype(np.float32)),
            "wk": np.ascontiguousarray(wk_b),
            "wv": np.ascontiguousarray(wv_b.astype(np.float16)),
            "vmask": vmask,
            "qmask": np.ascontiguousarray(qm),
        })
    return in_maps


def _run(inputs, trace=False, mm_dtype_name="", tmpdir=None):
    from concourse.bass_utils import run_bass_kernel_spmd

    key = "v7"
    if key not in _CACHE:
        _CACHE[key] = _build()
    nc = _CACHE[key]

    in_maps = _prep_inputs(**inputs)
    res = run_bass_kernel_spmd(nc, in_maps, core_ids=list(range(NCORES)),
                               trace=trace, tmpdir=tmpdir)
    out = np.stack([res.results[i]["out"] for i in range(NCORES)], axis=0)
    return out.astype(np.float32), res


def kernel(Q_seq, K_seq, V_seq, Q_len, V_len, WQ, WK, WV):
    out, _ = _run(dict(Q_seq=Q_seq, K_seq=K_seq, V_seq=V_seq,
                       Q_len=Q_len, V_len=V_len, WQ=WQ, WK=WK, WV=WV))
    return out
